# revision 1
# baseline (speedup 1.0000x reference)
"""Trainium2 Bass kernel for a 4-layer decoder backbone (nn_DecoderBackbone).

Sharding: data-parallel over batch (2) x tensor-parallel over heads/inter (4).
Core c: tp rank r = c % 4, batch b = c // 4. Each core owns 4 query heads +
their 1 KV head (GQA group == TP shard), 1408 MLP intermediate dims, and the
matching col/row slices of the o/down projections. Row-parallel o/down
partials are AllReduced over the 4-core group of each batch element
(replica groups [[0,1,2,3],[4,5,6,7]]).

On-device layout: the residual stream lives feature-major (xT [HID, T]) in
SBUF so every matmul contraction dim sits on partitions. All matmuls run in
float32r (full PE rate, ~1e-4 relative rounding). RMSNorm weights are folded
into the following projection weights on the host; the per-token 1/rms scale
is applied at PSUM eviction. Attention scores are computed transposed
(s[tk, tq]) so the softmax denominator comes from a ones-vector matmul and no
transposes of the attention matrix are needed. Softmax skips max-subtraction
(scores are O(+-10) for unit-scale inputs; exp is safe in fp32).
"""
import sys

sys.path.insert(0, "/opt/trn_rl_repo")

import numpy as np

L, B, T, HID = 4, 2, 1024, 2048
NH, NKV, HD = 16, 4, 128
INTER = 5632
EPS = 1e-6
NCORES, TP = 8, 4
QH = NH // TP              # q heads per core (4)
IC = INTER // TP           # inter dims per core (1408)
NIT = IC // 128            # inter tiles (11)
KT = HID // 128            # hid tiles (16)
NCT = QH + 2               # qkv col tiles (6): q0..q3, k, v
HALF = T // 2
RG = [[0, 1, 2, 3], [4, 5, 6, 7]]

_CACHE = {}


def _build_program(with_bias, depth_mult=1, fake_coll=False):
    import concourse.bacc as bacc
    import concourse.tile as tile
    import concourse.mybir as mybir
    from contextlib import ExitStack

    F32 = mybir.dt.float32
    F32R = mybir.dt.float32r
    BF16 = mybir.dt.bfloat16
    AF = mybir.ActivationFunctionType
    OP = mybir.AluOpType

    nc = bacc.Bacc("TRN2", target_bir_lowering=False, debug=False,
                   num_devices=NCORES)

    XT = nc.dram_tensor("xt_in", [HID, T], F32R, kind="ExternalInput")
    WQKV = nc.dram_tensor("wqkv", [L, NCT, HID, 128], F32R, kind="ExternalInput")
    WO = nc.dram_tensor("wo", [L, QH * HD, HID], F32R, kind="ExternalInput")
    WGU = nc.dram_tensor("wgu", [L, NIT, HID, 256], F32R, kind="ExternalInput")
    WD = nc.dram_tensor("wd", [L, IC, HID], F32R, kind="ExternalInput")
    COST = nc.dram_tensor("cost", [HD, T], F32, kind="ExternalInput")
    SINST = nc.dram_tensor("sinst", [HD, T], F32, kind="ExternalInput")
    MASKS = nc.dram_tensor("masks", [4, 128, 512], F32, kind="ExternalInput")
    ONES = nc.dram_tensor("ones", [128, 1], F32R, kind="ExternalInput")
    IDT = nc.dram_tensor("idt", [128, 128], F32R, kind="ExternalInput")
    NRMW = nc.dram_tensor("nrmw", [128, KT], F32, kind="ExternalInput")
    EPST = nc.dram_tensor("epst", [1, 1], F32, kind="ExternalInput")
    if with_bias:
        QKVB = nc.dram_tensor("qkvb", [L, 128, NCT], F32, kind="ExternalInput")
    OXT = nc.dram_tensor("oxt", [HID, T], F32, kind="ExternalOutput")

    with tile.TileContext(nc) as tc, ExitStack() as top:
        persist = top.enter_context(tc.tile_pool(name="persist", bufs=1))
        dram = top.enter_context(tc.tile_pool(name="dram", bufs=2, space="DRAM"))

        xt = persist.tile([128, KT, T], F32R)
        nc.sync.dma_start(out=xt, in_=XT.ap().rearrange("(k p) t -> p k t", p=128))
        cost = persist.tile([128, T], F32)
        nc.sync.dma_start(out=cost, in_=COST.ap())
        sinst = persist.tile([128, T], F32)
        nc.sync.dma_start(out=sinst, in_=SINST.ap())
        masks = persist.tile([128, 4, 512], F32)
        nc.sync.dma_start(out=masks, in_=MASKS.ap().rearrange("o p f -> p o f"))
        ones = persist.tile([128, 1], F32R)
        nc.sync.dma_start(out=ones, in_=ONES.ap())
        idt = persist.tile([128, 128], F32R)
        nc.sync.dma_start(out=idt, in_=IDT.ap())
        nrmw = persist.tile([128, KT], F32)
        nc.sync.dma_start(out=nrmw, in_=NRMW.ap())
        epst = persist.tile([1, 1], F32)
        nc.sync.dma_start(out=epst, in_=EPST.ap())
        if with_bias:
            qkvb = persist.tile([128, L, NCT], F32)
            nc.sync.dma_start(out=qkvb, in_=QKVB.ap().rearrange("l p c -> p l c"))

        def residual_add(pool, bounce, h):
            # xt[:, :, h-half] += bounce (AllReduce output, [HID, HALF] DRAM)
            c0 = h * HALF
            for k in range(KT):
                ar = pool.tile([128, HALF], F32, name="ar", bufs=3)
                nc.sync.dma_start(out=ar, in_=bounce[k * 128:(k + 1) * 128, :])
                nc.vector.tensor_tensor(
                    out=xt[:, k, c0:c0 + HALF],
                    in0=xt[:, k, c0:c0 + HALF].bitcast(F32), in1=ar, op=OP.add)

        def norm_scale(pool, pool_ps, h, sbc):
            # sbc[:, h-half] = broadcast rsqrt(mean(x^2) + eps) per token
            c0 = h * HALF
            var = pool_ps.tile([1, HALF], F32, name="var", bufs=1)
            for k in range(KT):
                sq = pool.tile([128, HALF], F32R, name="sq", bufs=3)
                nc.vector.tensor_tensor(
                    out=sq, in0=xt[:, k, c0:c0 + HALF].bitcast(F32),
                    in1=xt[:, k, c0:c0 + HALF].bitcast(F32), op=OP.mult)
                nc.tensor.matmul(var, ones, sq, start=(k == 0),
                                 stop=(k == KT - 1), skip_group_check=True)
            std = pool.tile([1, HALF], F32, name="std", bufs=1)
            nc.scalar.activation(out=std, in_=var, func=AF.Sqrt,
                                 bias=epst[:, 0:1], scale=1.0 / HID)
            rec = pool.tile([1, HALF], F32, name="rec", bufs=1)
            nc.vector.reciprocal(out=rec, in_=std)
            nc.gpsimd.partition_broadcast(sbc[:, c0:c0 + HALF], rec)

        pending_m = None  # previous layer's mlp AllReduce outputs, per half

        for l in [li % L for li in range(L * depth_mult)]:
            with ExitStack() as ls:
                sbL = ls.enter_context(tc.tile_pool(name="sbL", bufs=1))
                qf = sbL.tile([128, QH, T], F32R, name="qf")
                kf = sbL.tile([128, T], F32R, name="kf")
                vv = sbL.tile([128, 8, HD], F32R, name="vv")
                aoT = sbL.tile([128, QH, T], F32R, name="aoT")

                # ---------- phase A: residual + norm1 + qkv + rope ----------
                with ExitStack() as ph:
                    sbA = ph.enter_context(tc.tile_pool(name="sbA", bufs=2))
                    psW = ph.enter_context(tc.tile_pool(name="psW", bufs=3, space="PSUM"))
                    psV = ph.enter_context(tc.tile_pool(name="psV", bufs=2, space="PSUM"))
                    psS = ph.enter_context(tc.tile_pool(name="psS", bufs=1, space="PSUM"))

                    s1 = sbA.tile([128, T], F32, name="s1", bufs=1)
                    for h in (0, 1):
                        c0 = h * HALF
                        if pending_m is not None:
                            residual_add(sbA, pending_m[h], h)
                        norm_scale(sbA, psS, h, s1)
                        for ct in range(NCT):
                            w = sbA.tile([128, KT, 128], F32R, name="wq", bufs=3)
                            nc.sync.dma_start(
                                out=w,
                                in_=WQKV.ap()[l, ct].rearrange("(k p) c -> p k c", p=128))
                            ps = psW.tile([128, HALF], F32, name="pqkv", bufs=3)
                            for k in range(KT):
                                nc.tensor.matmul(ps, w[:, k, :], xt[:, k, c0:c0 + HALF],
                                                 start=(k == 0), stop=(k == KT - 1),
                                                 skip_group_check=True)
                            if ct == NCT - 1:  # v: evict straight to f32r + transpose
                                vtmp = sbA.tile([128, HALF], F32R, name="vtmp", bufs=2)
                                nc.vector.tensor_tensor(out=vtmp, in0=ps,
                                                        in1=s1[:, c0:c0 + HALF], op=OP.mult)
                                if with_bias:
                                    nc.vector.tensor_scalar_add(
                                        out=vtmp, in0=vtmp.bitcast(F32),
                                        scalar1=qkvb[:, l, ct:ct + 1])
                                for j in range(4):
                                    pv = psV.tile([128, 128], F32R, name="pv", bufs=2)
                                    nc.tensor.transpose(
                                        pv, vtmp[:, j * 128:(j + 1) * 128], idt)
                                    nc.scalar.copy(vv[:, h * 4 + j, :], pv.bitcast(F32))
                            else:  # q/k: evict to f32, rope, write f32r
                                qt = sbA.tile([128, HALF], F32, name="qt", bufs=2)
                                nc.vector.tensor_tensor(out=qt, in0=ps,
                                                        in1=s1[:, c0:c0 + HALF], op=OP.mult)
                                if with_bias:
                                    nc.vector.tensor_scalar_add(
                                        out=qt, in0=qt, scalar1=qkvb[:, l, ct:ct + 1])
                                rot = sbA.tile([128, HALF], F32, name="rot", bufs=2)
                                nc.sync.dma_start(out=rot[0:64, :], in_=qt[64:128, :])
                                nc.sync.dma_start(out=rot[64:128, :], in_=qt[0:64, :])
                                qc = sbA.tile([128, HALF], F32, name="qc", bufs=2)
                                nc.vector.tensor_tensor(out=qc, in0=qt,
                                                        in1=cost[:, c0:c0 + HALF], op=OP.mult)
                                nc.vector.tensor_tensor(out=rot, in0=rot,
                                                        in1=sinst[:, c0:c0 + HALF], op=OP.mult)
                                dst = (qf[:, ct, c0:c0 + HALF] if ct < QH
                                       else kf[:, c0:c0 + HALF])
                                nc.vector.tensor_tensor(out=dst, in0=qc, in1=rot, op=OP.add)

                # ---------- phase B: attention + o-proj + AllReduce ----------
                bounce_a = []
                with ExitStack() as ph:
                    sbB = ph.enter_context(tc.tile_pool(name="sbB", bufs=2))
                    with ExitStack() as pha:
                        psSc = pha.enter_context(tc.tile_pool(name="psSc", bufs=3, space="PSUM"))
                        psAO = pha.enter_context(tc.tile_pool(name="psAO", bufs=2, space="PSUM"))
                        psSum = pha.enter_context(tc.tile_pool(name="psSum", bufs=2, space="PSUM"))
                        for c in (0, 1):
                            t0 = c * HALF
                            nk = 4 * c + 4
                            for hh in range(QH):
                                pao = psAO.tile([128, HALF], F32, name="pao", bufs=2)
                                psm = psSum.tile([1, HALF], F32, name="psm", bufs=2)
                                for k in range(nk):
                                    sc = psSc.tile([128, HALF], F32, name="sc", bufs=3)
                                    nc.tensor.matmul(sc, kf[:, k * 128:(k + 1) * 128],
                                                     qf[:, hh, t0:t0 + HALF],
                                                     start=True, stop=True,
                                                     skip_group_check=True)
                                    ex = sbB.tile([128, HALF], F32R, name="ex", bufs=3)
                                    nc.scalar.activation(out=ex, in_=sc, func=AF.Exp)
                                    o = k - 4 * c
                                    if o >= 0:  # diagonal block: causal 0/1 mask
                                        nc.vector.tensor_tensor(
                                            out=ex, in0=ex.bitcast(F32),
                                            in1=masks[:, o, :], op=OP.mult)
                                    nc.tensor.matmul(pao, vv[:, k, :], ex,
                                                     start=(k == 0), stop=(k == nk - 1),
                                                     skip_group_check=True)
                                    nc.tensor.matmul(psm, ones, ex,
                                                     start=(k == 0), stop=(k == nk - 1),
                                                     skip_group_check=True)
                                rw = sbB.tile([1, HALF], F32, name="rw", bufs=2)
                                nc.vector.reciprocal(out=rw, in_=psm)
                                rb = sbB.tile([128, HALF], F32, name="rb", bufs=2)
                                nc.gpsimd.partition_broadcast(rb, rw)
                                nc.vector.tensor_tensor(out=aoT[:, hh, t0:t0 + HALF],
                                                        in0=pao, in1=rb, op=OP.mult)

                    # o-proj: partial over local heads -> AllReduce per half
                    psO = ph.enter_context(tc.tile_pool(name="psO", bufs=2, space="PSUM"))
                    wo_tiles = []
                    for kk in range(4):
                        wo_t = sbB.tile([128, HID], F32R, name=f"wo{kk}", bufs=1)
                        nc.sync.dma_start(out=wo_t,
                                          in_=WO.ap()[l][kk * 128:(kk + 1) * 128, :])
                        wo_tiles.append(wo_t)
                    for h in (0, 1):
                        c0 = h * HALF
                        bin_ = dram.tile([HID, HALF], F32, name="ba_i", bufs=2)
                        bout = dram.tile([HID, HALF], F32, name="ba_o", bufs=2)
                        for ho in range(KT):
                            po = psO.tile([128, HALF], F32, name="po", bufs=2)
                            for kk in range(4):
                                nc.tensor.matmul(po, wo_tiles[kk][:, ho * 128:(ho + 1) * 128],
                                                 aoT[:, kk, c0:c0 + HALF],
                                                 start=(kk == 0), stop=(kk == 3),
                                                 skip_group_check=True)
                            osb = sbB.tile([128, HALF], F32, name="osb", bufs=3)
                            nc.scalar.copy(osb, po)
                            nc.sync.dma_start(out=bin_[ho * 128:(ho + 1) * 128, :], in_=osb)
                        if fake_coll:
                            nc.sync.dma_start(out=bout, in_=bin_)
                        else:
                            nc.gpsimd.collective_compute(
                                "AllReduce", mybir.AluOpType.add, replica_groups=RG,
                                ins=[bin_.opt()], outs=[bout.opt()])
                        bounce_a.append(bout)

                # ---------- phase C: residual + norm2 + mlp + AllReduce ----------
                # halves outer: mT is per-half (SBUF), weights reloaded per
                # half; AllReduce(h0) overlaps the h1 compute.
                pending_m = []
                with ExitStack() as ph:
                    sbC = ph.enter_context(tc.tile_pool(name="sbC", bufs=2))
                    mTp = ph.enter_context(tc.tile_pool(name="mTp", bufs=1))
                    psG = ph.enter_context(tc.tile_pool(name="psG", bufs=1, space="PSUM"))
                    psU = ph.enter_context(tc.tile_pool(name="psU", bufs=1, space="PSUM"))
                    psD = ph.enter_context(tc.tile_pool(name="psD", bufs=2, space="PSUM"))
                    s2 = sbC.tile([128, T], F32, name="s2", bufs=1)
                    for h in (0, 1):
                        c0 = h * HALF
                        with ExitStack() as phv:
                            sbN = phv.enter_context(tc.tile_pool(name="sbN", bufs=2))
                            psS2 = phv.enter_context(
                                tc.tile_pool(name="psS2", bufs=1, space="PSUM"))
                            residual_add(sbN, bounce_a[h], h)
                            norm_scale(sbN, psS2, h, s2)
                        mT = mTp.tile([128, NIT, HALF], F32R, name="mT", bufs=1)
                        for ci in range(NIT):
                            pg = psG.tile([128, HALF], F32, name="pg", bufs=1)
                            pu = psU.tile([128, HALF], F32, name="pu", bufs=1)
                            for kh in (0, 1):  # stream weight K in halves
                                wgu = sbC.tile([128, 8, 256], F32R, name="wgu", bufs=2)
                                nc.sync.dma_start(
                                    out=wgu,
                                    in_=WGU.ap()[l, ci][kh * 1024:(kh + 1) * 1024, :]
                                    .rearrange("(k p) c -> p k c", p=128))
                                for k in range(8):
                                    kk = kh * 8 + k
                                    nc.tensor.matmul(
                                        pg, wgu[:, k, 0:128], xt[:, kk, c0:c0 + HALF],
                                        start=(kk == 0), stop=(kk == KT - 1),
                                        skip_group_check=True)
                                for k in range(8):
                                    kk = kh * 8 + k
                                    nc.tensor.matmul(
                                        pu, wgu[:, k, 128:256], xt[:, kk, c0:c0 + HALF],
                                        start=(kk == 0), stop=(kk == KT - 1),
                                        skip_group_check=True)
                            gev = sbC.tile([128, HALF], F32, name="gev", bufs=2)
                            nc.vector.tensor_tensor(out=gev, in0=pg,
                                                    in1=s2[:, c0:c0 + HALF], op=OP.mult)
                            gsl = sbC.tile([128, HALF], F32, name="gsl", bufs=2)
                            nc.scalar.activation(out=gsl, in_=gev, func=AF.Silu)
                            uev = sbC.tile([128, HALF], F32, name="uev", bufs=2)
                            nc.vector.tensor_tensor(out=uev, in0=pu,
                                                    in1=s2[:, c0:c0 + HALF], op=OP.mult)
                            nc.vector.tensor_tensor(out=mT[:, ci, :], in0=gsl, in1=uev,
                                                    op=OP.mult)
                        # down-proj partials for this half -> AllReduce
                        bm_i = dram.tile([HID, HALF], F32, name="bm_i", bufs=2)
                        for ho in range(KT):
                            wd_t = sbC.tile([128, NIT, 128], F32R, name="wd_t", bufs=2)
                            nc.sync.dma_start(
                                out=wd_t,
                                in_=WD.ap()[l][:, ho * 128:(ho + 1) * 128].rearrange(
                                    "(ki p) c -> p ki c", p=128))
                            pd = psD.tile([128, HALF], F32, name="pd", bufs=2)
                            for ki in range(NIT):
                                nc.tensor.matmul(pd, wd_t[:, ki, :],
                                                 mT[:, ki, :],
                                                 start=(ki == 0), stop=(ki == NIT - 1),
                                                 skip_group_check=True)
                            dsb = sbC.tile([128, HALF], F32, name="dsb", bufs=3)
                            nc.scalar.copy(dsb, pd)
                            nc.sync.dma_start(
                                out=bm_i[ho * 128:(ho + 1) * 128, :], in_=dsb)
                        bout = dram.tile([HID, HALF], F32, name="bm_o", bufs=2)
                        if fake_coll:
                            nc.sync.dma_start(out=bout, in_=bm_i)
                        else:
                            nc.gpsimd.collective_compute(
                                "AllReduce", mybir.AluOpType.add, replica_groups=RG,
                                ins=[bm_i.opt()], outs=[bout.opt()])
                        pending_m.append(bout)

        # ---------------- final norm + output ----------------
        with ExitStack() as ph:
            sbF = ph.enter_context(tc.tile_pool(name="sbF", bufs=2))
            psF = ph.enter_context(tc.tile_pool(name="psF", bufs=1, space="PSUM"))
            sf = sbF.tile([128, T], F32, name="sf", bufs=1)
            for h in (0, 1):
                residual_add(sbF, pending_m[h], h)
                norm_scale(sbF, psF, h, sf)
                c0 = h * HALF
                for k in range(KT):
                    tmp = sbF.tile([128, HALF], F32, name="tmp", bufs=3)
                    nc.vector.tensor_tensor(out=tmp,
                                            in0=xt[:, k, c0:c0 + HALF].bitcast(F32),
                                            in1=sf[:, c0:c0 + HALF], op=OP.mult)
                    ot = sbF.tile([128, HALF], F32, name="ot", bufs=3)
                    nc.vector.tensor_scalar_mul(out=ot, in0=tmp,
                                                scalar1=nrmw[:, k:k + 1])
                    nc.sync.dma_start(
                        out=OXT.ap()[k * 128:(k + 1) * 128, c0:c0 + HALF], in_=ot)

    nc.compile()
    return nc


def _prepare_inputs(inputs):
    g = {k: np.ascontiguousarray(np.asarray(v, dtype=np.float32))
         for k, v in inputs.items()}
    qw, kw, vw, ow = g["qw"], g["kw"], g["vw"], g["ow"]
    gatew, upw, downw = g["gatew"], g["upw"], g["downw"]
    ln1w, ln2w, normw = g["ln1w"], g["ln2w"], g["normw"]
    hs, cos, sin = g["hidden_states"], g["cos"], g["sin"]
    qb, kb, vb = g["qb"], g["kb"], g["vb"]

    with_bias = bool(np.any(qb) or np.any(kb) or np.any(vb))
    sc = 1.0 / np.sqrt(HD)

    shards = []
    for r in range(TP):
        sh = {
            "wqkv": np.empty([L, NCT, HID, 128], np.float32),
            "wo": np.empty([L, QH * HD, HID], np.float32),
            "wgu": np.empty([L, NIT, HID, 256], np.float32),
            "wd": np.empty([L, IC, HID], np.float32),
        }
        if with_bias:
            sh["qkvb"] = np.zeros([L, 128, NCT], np.float32)
        shards.append(sh)
    for l in range(L):
        qs = qw[l] * ln1w[l][None, :] * sc
        ks = kw[l] * ln1w[l][None, :]
        vs = vw[l] * ln1w[l][None, :]
        gs = gatew[l] * ln2w[l][None, :]
        us = upw[l] * ln2w[l][None, :]
        for r in range(TP):
            sh = shards[r]
            for j in range(QH):
                r0 = r * QH * HD + j * HD
                sh["wqkv"][l, j] = qs[r0:r0 + HD, :].T
                if with_bias:
                    sh["qkvb"][l, :, j] = qb[l, r0:r0 + HD] * sc
            sh["wqkv"][l, QH] = ks[r * HD:(r + 1) * HD, :].T
            sh["wqkv"][l, QH + 1] = vs[r * HD:(r + 1) * HD, :].T
            if with_bias:
                sh["qkvb"][l, :, QH] = kb[l, r * HD:(r + 1) * HD]
                sh["qkvb"][l, :, QH + 1] = vb[l, r * HD:(r + 1) * HD]
            sh["wo"][l] = ow[l][:, r * QH * HD:(r + 1) * QH * HD].T
            for ci in range(NIT):
                r0 = r * IC + ci * 128
                sh["wgu"][l, ci, :, 0:128] = gs[r0:r0 + 128, :].T
                sh["wgu"][l, ci, :, 128:256] = us[r0:r0 + 128, :].T
            sh["wd"][l] = downw[l][:, r * IC:(r + 1) * IC].T

    cost = cos[0].T.copy()                       # [HD, T]
    sinst = np.concatenate([-sin[0, :, :HD // 2].T, sin[0, :, HD // 2:].T], axis=0)
    p = np.arange(128)[:, None]
    f = np.arange(512)[None, :]
    masks = np.stack([(f >= 128 * o + p).astype(np.float32) for o in range(4)])
    common = {
        "cost": np.ascontiguousarray(cost),
        "sinst": np.ascontiguousarray(sinst),
        "masks": masks,
        "ones": np.ones([128, 1], np.float32),
        "idt": np.eye(128, dtype=np.float32),
        "nrmw": np.ascontiguousarray(normw.reshape(KT, 128).T),
        "epst": np.full([1, 1], EPS, np.float32),
    }
    in_maps = []
    for c in range(NCORES):
        r, b = c % TP, c // TP
        m = dict(common)
        m.update(shards[r])
        m["xt_in"] = np.ascontiguousarray(hs[b].T)
        in_maps.append(m)
    return in_maps, with_bias


def _get_program(with_bias, depth_mult=1, fake_coll=False):
    key = ("prog", with_bias, depth_mult, fake_coll)
    if key not in _CACHE:
        _CACHE[key] = _build_program(with_bias, depth_mult, fake_coll)
    return _CACHE[key]


def kernel(**inputs):
    from concourse import bass_utils
    in_maps, with_bias = _prepare_inputs(inputs)
    nc = _get_program(with_bias)
    r = bass_utils.run_bass_kernel_spmd(nc, in_maps,
                                        core_ids=list(range(NCORES)))
    out = np.stack([r.results[0]["oxt"].T, r.results[TP]["oxt"].T])
    return np.ascontiguousarray(out)

